# revision 11
# baseline (speedup 1.0000x reference)
"""Trainium2 Bass kernel for nn_HW_att_12893491822789.

Math (per sample):
  p[j]   = mean_c x[c,j] + max_c x[c,j]            (j indexes h*w = 2304)
  q_i    = Wq*p_i + bq ; k_j = Wk*p_j + bk
  attn   = softmax_j(q_i * k_j)                    (rank-1 energy)
  out    = attn @ (x^T Wv^T + bv)                  ([2304, 32])
  y      = gamma * out^T + x

Key identities used:
  softmax_j(q_i k_j) == softmax_j(t_i p_j) with t = (Wq*Wk)*p + bq*Wk
  (the q_i*bk term is constant along j and cancels; bk is unused).
  t_i p_j = p_i * (alpha p_j) + (beta p_j), so the whole energy row block
  E^T[j, i] = exp(p_i * sc_j + bi_j) is ONE scalar-engine activation per
  128-row tile (per-partition scale/bias APs), output in bf16.
  Z_i (softmax denom) comes for free as a ones-column in the V matmul.
  gamma is folded into the V weights on the host.

Values are tiny (t*p in [-0.3, 2.1] for this data distribution) so the
max-subtraction inside softmax is safely skipped (pure exp, no overflow).

Sharding: data-parallel over batch n: 32 samples -> 8 cores x 4 samples.
"""

import os
import sys

import numpy as np

for _p in ("/opt/trn_rl_repo", "/root/.axon_site/_ro/trn_rl_repo"):
    if os.path.isdir(_p) and _p not in sys.path:
        sys.path.append(_p)

import concourse.bacc as bacc
import concourse.bass as bass
import concourse.tile as tile
from concourse import bass_isa, mybir
from concourse.bass import ts
from concourse.bass_utils import run_bass_kernel_spmd

F32 = mybir.dt.float32
BF16 = mybir.dt.bfloat16

N_CORES = 8
NS = 4           # samples per core
C = 32           # channels
HW = 48 * 48     # 2304 spatial positions
NJT = HW // 128  # 18 tiles of 128 along hw
CHUNK = 512      # psum-bank-sized matmul chunk (512 f32 = 2 KB = 1 bank)
CHUNKS = [(ch, min(CHUNK, HW - ch)) for ch in range(0, HW, CHUNK)]


def _kernel_body(tc, yout, xin, wb, ab):
    nc = tc.nc
    from contextlib import ExitStack

    from concourse.masks import make_identity

    with ExitStack() as ctx:
        consts = ctx.enter_context(tc.tile_pool(name="consts", bufs=1))
        xo_pool = ctx.enter_context(tc.tile_pool(name="xo", bufs=2))
        xb_pool = ctx.enter_context(tc.tile_pool(name="xbf", bufs=2))
        ar_pool = ctx.enter_context(tc.tile_pool(name="ar", bufs=2))
        p_pool = ctx.enter_context(tc.tile_pool(name="prow", bufs=2))
        pb_pool = ctx.enter_context(tc.tile_pool(name="pbcast", bufs=2))
        st_pool = ctx.enter_context(tc.tile_pool(name="stacks", bufs=2))
        e_pool = ctx.enter_context(tc.tile_pool(name="etile", bufs=4))
        u_pool = ctx.enter_context(tc.tile_pool(name="utile", bufs=4))
        vm_pool = ctx.enter_context(tc.tile_pool(name="vmat", bufs=2))
        z_pool = ctx.enter_context(tc.tile_pool(name="znorm", bufs=2))
        at_pool = ctx.enter_context(tc.tile_pool(name="attnrm", bufs=2))
        y_pool = ctx.enter_context(tc.tile_pool(name="ytile", bufs=2))
        ps_small = ctx.enter_context(tc.tile_pool(name="ps_small", bufs=2, space="PSUM"))
        ps_acc = ctx.enter_context(tc.tile_pool(name="ps_acc", bufs=1, space="PSUM"))
        dram_scratch = ctx.enter_context(tc.tile_pool(name="dscratch", bufs=2, space="DRAM"))

        # constants
        wb_sb = consts.tile([33, C], F32)
        nc.sync.dma_start(out=wb_sb, in_=wb)
        wb_bf = consts.tile([33, C], BF16)
        nc.vector.tensor_copy(wb_bf, wb_sb)
        ab_bc = consts.tile([128, 2], F32)
        nc.sync.dma_start(out=ab_bc, in_=ab.to_broadcast((128, 2)))
        ident32 = consts.tile([32, 32], F32)
        make_identity(nc, ident32)

        for s in range(NS):
            # x for this sample + a ones row (feeds the bias row of WB)
            xo = xo_pool.tile([C + 1, HW], F32)
            nc.sync.dma_start(out=xo[0:C, :], in_=xin[s])
            nc.gpsimd.memset(xo[C : C + 1, :], 1.0)
            xbf = xb_pool.tile([C + 1, HW], BF16)
            nc.vector.tensor_copy(xbf, xo)

            # pooling over channels: sum and max across the 32 partitions
            pr_s = ar_pool.tile([C, HW], F32, tag="pr_s")
            pr_m = ar_pool.tile([C, HW], F32, tag="pr_m")
            nc.gpsimd.partition_all_reduce(
                pr_s, xo[0:C, :], channels=C, reduce_op=bass_isa.ReduceOp.add
            )
            nc.gpsimd.partition_all_reduce(
                pr_m, xo[0:C, :], channels=C, reduce_op=bass_isa.ReduceOp.max
            )
            # p = sum/32 + max  (row 0 is enough)
            p_row = p_pool.tile([1, HW], F32)
            nc.vector.tensor_scalar(
                out=p_row,
                in0=pr_s[0:1, :],
                scalar1=1.0 / C,
                scalar2=None,
                op0=mybir.AluOpType.mult,
            )
            nc.vector.tensor_add(p_row, p_row, pr_m[0:1, :])

            # broadcast p across all 128 partitions (ACT input operand)
            p_bc = pb_pool.tile([128, HW], F32)
            nc.gpsimd.partition_broadcast(p_bc, p_row, channels=128)

            # per-partition copy of p: [128, 18] where col jt holds p[jt*128 + r].
            # SBUF->SBUF partition<->free flips don't balance, so bounce via DRAM.
            p_dram = dram_scratch.tile([1, HW], F32, tag="p_dram")
            nc.sync.dma_start(out=p_dram, in_=p_row)
            ps_t = st_pool.tile([128, NJT], F32, tag="ps_t")
            nc.sync.dma_start(
                out=ps_t, in_=p_dram.rearrange("o (t j) -> o j t", j=128)
            )
            # scale = alpha*p_j, bias = beta*p_j  (per-partition operands of exp)
            sc_st = st_pool.tile([128, NJT], F32, tag="sc_st")
            bi_st = st_pool.tile([128, NJT], F32, tag="bi_st")
            nc.vector.tensor_scalar(
                out=sc_st, in0=ps_t, scalar1=ab_bc[:, 0:1], scalar2=None,
                op0=mybir.AluOpType.mult,
            )
            nc.vector.tensor_scalar(
                out=bi_st, in0=ps_t, scalar1=ab_bc[:, 1:2], scalar2=None,
                op0=mybir.AluOpType.mult,
            )

            # V[o, j] = gamma*(Wv x + bv)[o, j] on PE, then bounce through DRAM
            # so U tiles can be loaded in the reference's .view() layout:
            # U[j, d] = V_flat[32*j + d].
            vm_sb = vm_pool.tile([C, HW], BF16)
            for ch, cw in CHUNKS:
                vps = ps_small.tile([C, CHUNK], F32, tag="small")
                nc.tensor.matmul(
                    vps[:, 0:cw], lhsT=wb_bf, rhs=xbf[:, ch : ch + cw],
                    start=True, stop=True,
                )
                nc.vector.tensor_copy(vm_sb[:, ch : ch + cw], vps[:, 0:cw])
            vm_dram = dram_scratch.tile([C, HW], BF16, tag="vm_dram")
            nc.sync.dma_start(out=vm_dram, in_=vm_sb)
            u_view = vm_dram.rearrange("c (m d) -> (c m) d", d=C)  # [HW, 32]

            # accumulator: rows 0..31 = gamma*(attn_un @ v_r)^T, row 32 = Z
            acc = ps_acc.tile([C + 1, HW], F32)

            for jt in range(NJT):
                # E^T[j, i] = exp(p_i * sc_j + bi_j)  -- one ACT instruction
                e_t = e_pool.tile([128, HW], BF16)
                nc.scalar.activation(
                    out=e_t,
                    in_=p_bc,
                    func=mybir.ActivationFunctionType.Exp,
                    bias=bi_st[:, jt : jt + 1],
                    scale=sc_st[:, jt : jt + 1],
                )
                # U tile: rows j of reshaped V, plus a ones column for Z
                u_t = u_pool.tile([128, C + 1], BF16)
                nc.sync.dma_start(out=u_t[:, 0:C], in_=u_view[ts(jt, 128), :])
                nc.gpsimd.memset(u_t[:, C : C + 1], 1.0)
                # acc[d, i] += sum_j U[j, d] * E^T[j, i]
                for ch, cw in CHUNKS:
                    nc.tensor.matmul(
                        acc[:, ch : ch + cw],
                        lhsT=u_t,
                        rhs=e_t[:, ch : ch + cw],
                        start=(jt == 0),
                        stop=(jt == NJT - 1),
                    )

            # normalize by Z: att^T[d, i] = acc[d, i] / Z_i  (gamma pre-folded)
            zrec = z_pool.tile([1, HW], F32, tag="zrec")
            nc.vector.reciprocal(zrec, acc[C : C + 1, :])
            zb = z_pool.tile([C, HW], F32, tag="zb")
            nc.gpsimd.partition_broadcast(zb, zrec, channels=C)
            att_sb = at_pool.tile([C, HW], F32)
            nc.vector.tensor_mul(att_sb, acc[0:C, :], zb)

            # out matrix is [i, d] row-major in the reference; transpose att^T
            # tile-by-tile on PE and store rows to DRAM, then reload the
            # .view(n, c, s, h, w) interpretation (contiguous) and add residual.
            om_dram = dram_scratch.tile([HW, C], F32, tag="om_dram")
            for it in range(NJT):
                tp = ps_small.tile([128, C], F32, tag="small")
                nc.tensor.transpose(tp, att_sb[:, ts(it, 128)], ident32)
                tp_sb = u_pool.tile([128, C], F32, tag="tp_sb")
                nc.vector.tensor_copy(tp_sb, tp)
                nc.sync.dma_start(out=om_dram[ts(it, 128), :], in_=tp_sb)
            y_sb = y_pool.tile([C, HW], F32)
            nc.sync.dma_start(
                out=y_sb, in_=om_dram.rearrange("(c m) d -> c (m d)", c=C)
            )
            nc.vector.tensor_add(y_sb, y_sb, xo[0:C, :])
            nc.sync.dma_start(out=yout[s], in_=y_sb)


def _install_ntff_hook():
    """Register the axon NTFF profiling hook if the image lacks
    antenv.axon_hooks (profiling-only; harmless when absent)."""
    import sys as _sys
    import types as _types

    if "antenv.axon_hooks" in _sys.modules:
        return
    try:
        import antenv.axon_hooks  # noqa: F401

        return
    except ImportError:
        pass
    try:
        from trn_agent_boot.trn_boot import _ntff_profile_via_ctypes
    except ImportError:
        return
    so = "/opt/axon/libaxon_pjrt.so"
    if not os.path.exists(so):
        return
    try:
        hook = _ntff_profile_via_ctypes(so)
    except OSError:
        return
    m = _types.ModuleType("antenv.axon_hooks")
    m.get_axon_ntff_profile_hook = lambda: hook
    m.set_axon_ntff_profile_hook = lambda h: None
    _sys.modules["antenv.axon_hooks"] = m


_NC = None


def _get_nc():
    global _NC
    if _NC is None:
        nc = bacc.Bacc(
            "TRN2",
            target_bir_lowering=False,
            debug=False,
            enable_asserts=False,
            num_devices=N_CORES,
        )
        xin = nc.dram_tensor("xin", [NS, C, HW], F32, kind="ExternalInput").ap()
        wb = nc.dram_tensor("wb", [33, C], F32, kind="ExternalInput").ap()
        ab = nc.dram_tensor("ab", [1, 2], F32, kind="ExternalInput").ap()
        yout = nc.dram_tensor("yout", [NS, C, HW], F32, kind="ExternalOutput").ap()
        with tile.TileContext(nc) as tc:
            _kernel_body(tc, yout, xin, wb, ab)
        nc.compile()
        _NC = nc
    return _NC


def kernel(x, Wq, bq, Wk, bk, Wv, bv, gamma):
    x = np.asarray(x, dtype=np.float32)
    Wq = np.asarray(Wq, dtype=np.float32)
    bq = np.asarray(bq, dtype=np.float32)
    Wk = np.asarray(Wk, dtype=np.float32)
    Wv = np.asarray(Wv, dtype=np.float32)
    bv = np.asarray(bv, dtype=np.float32)
    gamma = np.asarray(gamma, dtype=np.float32)

    n, c, s, h, w = x.shape
    assert (n, c, s, h * w) == (N_CORES * NS, C, 1, HW)

    g = np.float32(gamma[0])
    # WB[c', o] = gamma * Wv[o, c'] with a bias row (x is augmented with ones)
    WB = np.zeros((33, C), dtype=np.float32)
    WB[0:C, 0:C] = g * Wv.T
    WB[C, 0:C] = g * bv
    alpha = np.float32(Wq[0, 0]) * np.float32(Wk[0, 0])
    beta = np.float32(bq[0]) * np.float32(Wk[0, 0])
    ab = np.array([[alpha, beta]], dtype=np.float32)

    xr = x.reshape(n, C, HW)
    in_maps = [
        {"xin": np.ascontiguousarray(xr[core * NS : (core + 1) * NS]), "wb": WB, "ab": ab}
        for core in range(N_CORES)
    ]

    nc = _get_nc()
    if os.environ.get("BASS_TRACE"):
        _install_ntff_hook()
    res = run_bass_kernel_spmd(nc, in_maps, list(range(N_CORES)))
    kernel.last_results = res

    y = np.concatenate([res.results[core]["yout"] for core in range(N_CORES)], axis=0)
    return y.reshape(n, c, s, h, w).astype(np.float32)


kernel.last_results = None
